# revision 1
# baseline (speedup 1.0000x reference)
"""Self-contained Trainium2 kernel for nn_B3SplineUWT (3-level B3-spline
undecimated wavelet transform), data-parallel over 8 NeuronCores.

kernel(x: [8,1024,1024] f32) -> [8,4,1024,1024] f32  (w1,w2,w3,c3)

Per core: one image, bf16 internal data path (~5e-3 rel, tol 2e-2).
  - H-conv (partition dim): PE banded matmuls with raw integer taps
    (1,4,6), 3-block banded form; the 1/256 normalization (both
    separable passes) rides the ACT PSUM-evacuation scale for free.
  - W-conv: per-level split between PE (5 shifted accumulating matmuls,
    stationary w*I, the shift in the rhs AP offset) and DVE (4 chained
    (1 + z^d) bf16 tensor_adds -- the binomial factorization of the
    5-tap (1,4,6,4,1); even shifts for d=2,4 keep the DVE 2x mode).
  - w_j = c_{j-1} - c_j on DVE in bf16 into bf16 staging.
  - I/O via SWDGE cast-DMAs (f32->bf16 load, bf16->f32 store), streamed
    per 2 chunks (per chunk for c3) so the DMA drains during compute.
  - Per-chunk tiles, one buffer generation per level, 1-bank PSUM
    tiles, wavefront (level,chunk) emission order for cross-level
    pipelining.

Engine budget (cost model, per core): DVE ~52us (critical chain),
DMA ~55us, PE ~47us, ACT ~40us, Pool ~33us; end-to-end ~80us.
"""
import ml_dtypes
import numpy as np

import concourse.bacc as bacc
import concourse.bass as bass
import concourse.mybir as mybir
import concourse.tile as tile
from concourse.bass_utils import run_bass_kernel_spmd

F32 = mybir.dt.float32
BF16 = mybir.dt.bfloat16
COPY = mybir.ActivationFunctionType.Copy

B = 8
H = 1024
W = 1024
P = 128
NCH = H // P
LEVELS = 3
DILS = (1, 2, 4)
MARG = 16           # left/right margin in yx (>= 2*max(d) = 8, 4B-aligned)
WE = W + 2 * MARG

# chunks whose W-conv runs on PE (rest on DVE), per level; tuned by
# cost-model search so both engines stay busy through each level's tail
PE_ROUTE = ({6}, {0, 1, 3, 5, 6}, {1, 3, 4, 6})

YX_BUFS = 6
CASC_BUFS = 3
WST_BUFS = 4
PSUM_BUFS = 8       # H-conv psum slots; W-conv gets its own tag when
PC_BUFS = 2         # PC_BUFS > 0 (PT+PC banks must total <= 8)
WAVE_LAG = 4                  # chunk skew between consecutive levels
INPUT_GROUPS = (1, 1, 3, 3)   # chunks per input cast-DMA
W_GROUP = (2, 2, 2)           # chunks per w_j output DMA, per level
W3_VIA_ACCUM = False          # w3 = c2 - c3 via CCE accum DMAs

TAPS = {0: 6.0, 1: 4.0, 2: 1.0}   # raw integer taps, exact in bf16
EVAC_SCALE = 1.0 / 256.0          # both 1/16 normalizations, on ACT evac


def _reflect(i, n):
    if i < 0:
        return -i
    if i >= n:
        return 2 * (n - 1) - i
    return i


def _build_blocks():
    """Per level: diagonal 128x128 blocks D[co] (reflect folded at the
    edges) and off-diagonal neighbor blocks for the banded H-conv."""
    per_level = []
    for d in DILS:
        full = np.zeros((H, H), np.float32)
        for r in range(H):
            for o in (-2 * d, -d, 0, d, 2 * d):
                full[_reflect(r + o, H), r] += TAPS[abs(o) // d]
        dblk, offdiag = [], []
        for co in range(NCH):
            r0 = co * P
            dblk.append(np.ascontiguousarray(full[r0:r0 + P, r0:r0 + P]))
            od = []
            for ci in (co - 1, co + 1):
                if 0 <= ci < NCH:
                    blk = full[ci * P:(ci + 1) * P, r0:r0 + P]
                    if np.any(blk != 0):
                        od.append((ci, np.ascontiguousarray(blk)))
            offdiag.append(od)
        per_level.append((dblk, offdiag))
    return per_level


def _pack_consts(per_level):
    mats, seen = [], {}

    def intern(m):
        h = m.tobytes()
        if h not in seen:
            seen[h] = len(mats) * P
            mats.append(m)
        return seen[h]

    index = []
    for dblk, offdiag in per_level:
        doffs = [intern(m) for m in dblk]
        ooffs = [[(ci, intern(m)) for ci, m in od] for od in offdiag]
        index.append((doffs, ooffs))
    ident_offs = {}
    for w in (1.0, 4.0, 6.0):
        ident_offs[w] = len(mats) * P
        mats.append(np.eye(P, dtype=np.float32) * w)
    # host-side bf16 (all entries are small integers -- exact): lets the
    # consts ride a plain HWDGE DMA instead of a Pool-gen'd cast DMA on
    # the kernel's startup critical path
    packed = np.ascontiguousarray(
        np.concatenate(mats, axis=1).astype(ml_dtypes.bfloat16))
    return packed, index, ident_offs


def _build_program():
    per_level = _build_blocks()
    consts_np, cindex, ident_offs = _pack_consts(per_level)
    ncols_const = consts_np.shape[1]

    nc = bacc.Bacc("TRN2", target_bir_lowering=False, debug=False)
    x_d = nc.dram_tensor("x", [H, W], F32, kind="ExternalInput")
    c_d = nc.dram_tensor("consts", [P, ncols_const], BF16,
                         kind="ExternalInput")
    out_d = nc.dram_tensor("out", [LEVELS + 1, H, W], F32,
                           kind="ExternalOutput")

    with tile.TileContext(nc) as tc:
        with tc.tile_pool(name="sb", bufs=1) as sb, \
             tc.tile_pool(name="yxp", bufs=YX_BUFS) as yxp, \
             tc.tile_pool(name="casc", bufs=CASC_BUFS) as casc, \
             tc.tile_pool(name="wst", bufs=WST_BUFS) as wstp, \
             tc.tile_pool(name="ps", bufs=PSUM_BUFS, space="PSUM") as ps:

            # constants: bf16 DRAM -> bf16 SBUF via HWDGE (off Pool)
            cr = sb.tile([P, ncols_const], BF16, tag="cr", name="cr")
            nc.sync.dma_start(cr[:], c_d[:])

            def wident(w):
                off = ident_offs[w]
                return cr[:, off:off + P]

            # input: f32 DRAM -> bf16 SBUF cast DMAs; single-chunk loads
            # up front so the first H-convs (and the DVE cascade chain
            # behind them) start as early as possible
            xq = []          # per-chunk accessor: (tile, idx_in_tile)
            for g, n in enumerate(INPUT_GROUPS):
                t = sb.tile([P, n, W], BF16, tag=f"xq{g}", name=f"xq{g}")
                base = sum(INPUT_GROUPS[:g])
                nc.gpsimd.dma_start(
                    t[:],
                    bass.AP(x_d, base * P * W,
                            [[W, P], [P * W, n], [1, W]]))
                for k in range(n):
                    xq.append((t, k))

            # per-chunk level buffers, one generation per level (no WARs)
            cbuf = [[sb.tile([P, W], BF16, tag=f"c{g}_{co}",
                             name=f"c{g}_{co}") for co in range(NCH)]
                    for g in range(LEVELS)]

            def chunk_in(j, co, lo=0, hi=W):
                if j == 0:
                    t, k = xq[co]
                    return t[:, k, lo:hi]
                return cbuf[j - 1][co][:, lo:hi]

            def cur_ap(j, co, lo=0, hi=W):
                return cbuf[j][co][:, lo:hi]

            # wavefront emission order: level j trails level j-1 by
            # WAVE_LAG chunks, so late-level outputs stream from
            # mid-kernel instead of piling into a DMA-only tail
            order = []
            for wave in range(NCH + WAVE_LAG * (LEVELS - 1)):
                for j in range(LEVELS):
                    co = wave - WAVE_LAG * j
                    if 0 <= co < NCH:
                        order.append((j, co))

            wsts = {}
            for j, co in order:
                d = DILS[j]
                doffs, ooffs = cindex[j]
                shifts = [(0, 6.0), (-d, 4.0), (d, 4.0),
                          (-2 * d, 1.0), (2 * d, 1.0)]

                # ---- H-conv into PSUM (banded matmuls, raw taps) ----
                yx = yxp.tile([P, WE], BF16, tag="yx", name="yx")
                for half in range(2):
                    lo, hi = half * 512, (half + 1) * 512
                    pt = ps.tile([P, 512], F32, tag="psum", name="pt",
                                 bufs=PSUM_BUFS - PC_BUFS)
                    mms = ([(doffs[co], None)] +
                           [(off, ci) for ci, off in ooffs[co]])
                    for i, (off, ci) in enumerate(mms):
                        nc.tensor.matmul(
                            pt[:], cr[:, off:off + P],
                            chunk_in(j, co if ci is None else ci, lo, hi),
                            start=(i == 0),
                            stop=(i == len(mms) - 1))
                    # evacuate with the 1/256 scale into the margin tile
                    nc.scalar.activation(
                        yx[:, MARG + lo:MARG + hi],
                        pt[:], COPY, scale=EVAC_SCALE)

                # reflect margins (Pool): yx[M-k] = yx[M+k]
                nc.gpsimd.tensor_copy(
                    bass.AP(yx.tensor, MARG - 2 * d, [[WE, P], [1, 2 * d]]),
                    bass.AP(yx.tensor, MARG + 2 * d, [[WE, P], [-1, 2 * d]]))
                nc.gpsimd.tensor_copy(
                    bass.AP(yx.tensor, MARG + W, [[WE, P], [1, 2 * d]]),
                    bass.AP(yx.tensor, MARG + W - 2, [[WE, P], [-1, 2 * d]]))

                # ---- W-conv ----
                if co in PE_ROUTE[j]:
                    # PE route: 5 shifted accumulating matmuls per half
                    for half in range(2):
                        pc = ps.tile([P, 512], F32,
                                     tag="psumw" if PC_BUFS else "psum",
                                     name="pc",
                                     bufs=PC_BUFS or PSUM_BUFS)
                        base = MARG + half * 512
                        for i, (off, wgt) in enumerate(shifts):
                            nc.tensor.matmul(
                                pc[:],
                                wident(wgt),
                                bass.AP(yx.tensor, base + off,
                                        [[WE, P], [1, 512]]),
                                start=(i == 0),
                                stop=(i == len(shifts) - 1))
                        nc.scalar.copy(
                            cur_ap(j, co, half * 512, (half + 1) * 512),
                            pc[:])
                else:
                    # DVE route: 4 chained (1 + z^d) adds, bf16 2x mode
                    def yxs(o, width):
                        return bass.AP(yx.tensor, MARG + o,
                                       [[WE, P], [1, width]])
                    t1 = casc.tile([P, WE], BF16, tag="t1", name="t1")
                    t2 = casc.tile([P, WE], BF16, tag="t2", name="t2")
                    w1 = W + 3 * d
                    nc.vector.tensor_add(
                        t1[:, :w1], yxs(-2 * d, w1), yxs(-d, w1))
                    w2 = W + 2 * d
                    nc.vector.tensor_add(
                        t2[:, :w2], t1[:, :w2], t1[:, d:d + w2])
                    w3 = W + d
                    nc.vector.tensor_add(
                        t1[:, :w3], t2[:, :w3], t2[:, d:d + w3])
                    nc.vector.tensor_add(
                        cur_ap(j, co), t1[:, :W], t1[:, d:d + W])

                # c3: stream each chunk as soon as its W-conv is done
                if j == LEVELS - 1:
                    nc.gpsimd.dma_start(
                        bass.AP(out_d, 3 * H * W + co * P * W,
                                [[W, P], [1, W]]),
                        cur_ap(j, co))

                if j == LEVELS - 1 and W3_VIA_ACCUM:
                    # w3 = c2 - c3 entirely in the DMA engines: write the
                    # c2 chunk into the w3 region, then accum-subtract c3
                    w3ap = bass.AP(out_d, j * H * W + co * P * W,
                                   [[W, P], [1, W]])
                    nc.gpsimd.dma_start(w3ap, chunk_in(j, co))
                    nc.gpsimd.dma_start(
                        w3ap, cur_ap(j, co),
                        accum_op=mybir.AluOpType.subtract)
                    continue

                # ---- w_j = prev - cur (bf16) into staging ----
                wg = W_GROUP[j]
                hv, ci_ = divmod(co, wg)
                if ci_ == 0:
                    wsts[(j, hv)] = wstp.tile([P, wg, W], BF16,
                                              tag="wst", name="wst")
                nc.vector.tensor_sub(
                    wsts[(j, hv)][:, ci_, :], chunk_in(j, co),
                    cur_ap(j, co))

                # ---- stream out per group (cast bf16->f32) ----
                if ci_ == wg - 1:
                    nc.gpsimd.dma_start(
                        bass.AP(out_d, j * H * W + hv * wg * P * W,
                                [[W, P], [P * W, wg], [1, W]]),
                        wsts[(j, hv)][:])

    nc.compile()
    return nc, consts_np


_CACHE = {}


def _get_program():
    if "prog" not in _CACHE:
        _CACHE["prog"] = _build_program()
    return _CACHE["prog"]


def kernel(x, _trace=False, _trace_kwargs=None):
    """x: [8, 1024, 1024] float32 -> [8, 4, 1024, 1024] float32."""
    x = np.asarray(x)
    assert x.shape == (B, H, W) and x.dtype == np.float32
    nc, consts_np = _get_program()
    in_maps = [{"x": np.ascontiguousarray(x[b]), "consts": consts_np}
               for b in range(B)]
    kw = {}
    if _trace:
        kw = dict(trace=True, **(_trace_kwargs or {}))
    res = run_bass_kernel_spmd(nc, in_maps, core_ids=list(range(B)), **kw)
    out = np.stack([r["out"] for r in res.results], axis=0)
    if _trace:
        return out, res
    return out



# revision 2
# speedup vs baseline: 1.0048x; 1.0048x over previous
"""Trainium2 kernel v2 for nn_B3SplineUWT — scheduling-optimized.

Differences vs v1 baseline:
  - 1024-wide (2-bank) PSUM tiles: one ACT evac per chunk instead of two.
  - Reflect margins: ONE dual-region AP copy per chunk (on ACT for
    PE-routed chunks, DVE for DVE-routed — adjacent to the consumer);
    Pool does zero tensor work.
  - 1/256 normalization folded into the H-conv weights (exact in bf16).
  - c3 staged contiguously ([P, g, W] tiles) so its cast-store DMAs are
    grouped like the w stores — fewer Pool descriptor-gens.
  - Pool engine runs ONLY SWDGE descriptor-gen, in wavefront order.
"""
import ml_dtypes
import numpy as np

import concourse.bacc as bacc
import concourse.bass as bass
import concourse.mybir as mybir
import concourse.tile as tile
from concourse.bass_utils import run_bass_kernel_spmd

F32 = mybir.dt.float32
BF16 = mybir.dt.bfloat16
COPY = mybir.ActivationFunctionType.Copy

B = 8
H = 1024
W = 1024
P = 128
NCH = H // P
LEVELS = 3
DILS = (1, 2, 4)
MARG = 16
WE = W + 2 * MARG

# tunables (best timeline-sim config, est 69229)
PE_ROUTE = ({2, 6}, {0, 2, 4, 5}, {0, 3, 4, 6, 7})
YX_BUFS = 8
CASC_BUFS = 5
WST_BUFS = 4
C3_BUFS = 3
PT_BUFS = 4          # H psum tiles (1 bank each when WIDE_PSUM=False)
PC_BUFS = 4          # W psum tiles
WAVE_LAG = (3, 6)
INPUT_GROUPS = (1, 1, 1, 1, 2, 2)
W_GROUP = ((2, 2, 2, 1, 1), (1,) * 8, (1,) * 8)
C3_GROUP = (1, 1, 1, 1, 1, 1, 1, 1)

TAPS = {0: 6.0, 1: 4.0, 2: 1.0}
HSCALE = 1.0 / 256.0   # folded into H weights; exact in bf16


def _reflect(i, n):
    if i < 0:
        return -i
    if i >= n:
        return 2 * (n - 1) - i
    return i


def _build_blocks():
    per_level = []
    for d in DILS:
        full = np.zeros((H, H), np.float32)
        for r in range(H):
            for o in (-2 * d, -d, 0, d, 2 * d):
                full[_reflect(r + o, H), r] += TAPS[abs(o) // d]
        full *= HSCALE
        dblk, offdiag = [], []
        for co in range(NCH):
            r0 = co * P
            dblk.append(np.ascontiguousarray(full[r0:r0 + P, r0:r0 + P]))
            od = []
            for ci in (co - 1, co + 1):
                if 0 <= ci < NCH:
                    blk = full[ci * P:(ci + 1) * P, r0:r0 + P]
                    if np.any(blk != 0):
                        od.append((ci, np.ascontiguousarray(blk)))
            offdiag.append(od)
        per_level.append((dblk, offdiag))
    return per_level


def _pack_consts(per_level):
    mats, seen = [], {}

    def intern(m):
        h = m.tobytes()
        if h not in seen:
            seen[h] = len(mats) * P
            mats.append(m)
        return seen[h]

    index = []
    ident_offs = {}
    ncols_a = None
    for li, (dblk, offdiag) in enumerate(per_level):
        doffs = [intern(m) for m in dblk]
        ooffs = [[(ci, intern(m)) for ci, m in od] for od in offdiag]
        index.append((doffs, ooffs))
        if li == 0:
            # idents ride in part A with the level-0 blocks
            for w in (1.0, 4.0, 6.0):
                ident_offs[w] = len(mats) * P
                mats.append(np.eye(P, dtype=np.float32) * w)
            ncols_a = len(mats) * P
    packed = np.ascontiguousarray(
        np.concatenate(mats, axis=1).astype(ml_dtypes.bfloat16))
    return packed, index, ident_offs, ncols_a


def _build_program(**ov):
    g = dict(PE_ROUTE=PE_ROUTE, YX_BUFS=YX_BUFS, CASC_BUFS=CASC_BUFS,
             WST_BUFS=WST_BUFS, C3_BUFS=C3_BUFS, PT_BUFS=PT_BUFS,
             PC_BUFS=PC_BUFS, WAVE_LAG=WAVE_LAG,
             INPUT_GROUPS=INPUT_GROUPS, W_GROUP=W_GROUP,
             C3_GROUP=C3_GROUP, LEVEL_DESC=False, WIDE_PSUM=False,
             WARMUP_N=55, FAST_CHUNKS=0,
             POOL_SUBS=(set(), set(), set()),
             CASC1_POOL=(set(), set(), set()), STAGE_LAG=0)
    # normalize group specs: int n -> even partition [n,n,...]; list kept
    def _norm(gs):
        if isinstance(gs, int):
            out, left = [], NCH
            while left:
                out.append(min(gs, left))
                left -= out[-1]
            return tuple(out)
        return tuple(gs)
    g.update(ov)
    g['W_GROUP'] = tuple(_norm(x) for x in g['W_GROUP'])
    g['C3_GROUP'] = _norm(g['C3_GROUP'])
    return _build_program_p(**g)


def _build_program_p(PE_ROUTE, YX_BUFS, CASC_BUFS, WST_BUFS, C3_BUFS,
                     PT_BUFS, PC_BUFS, WAVE_LAG, INPUT_GROUPS, W_GROUP,
                     C3_GROUP, LEVEL_DESC, WIDE_PSUM, WARMUP_N,
                     FAST_CHUNKS, POOL_SUBS, CASC1_POOL, STAGE_LAG):
    per_level = _build_blocks()
    consts_np, cindex, ident_offs, ncols_a = _pack_consts(per_level)
    ncols_const = consts_np.shape[1]

    nc = bacc.Bacc("TRN2", target_bir_lowering=False, debug=False)
    x_d = nc.dram_tensor("x", [H, W], F32, kind="ExternalInput")
    c_d = nc.dram_tensor("consts", [P, ncols_const], BF16,
                         kind="ExternalInput")
    out_d = nc.dram_tensor("out", [LEVELS + 1, H, W], F32,
                           kind="ExternalOutput")

    with tile.TileContext(nc) as tc:
        with tc.tile_pool(name="sb", bufs=1) as sb, \
             tc.tile_pool(name="yxp", bufs=YX_BUFS) as yxp, \
             tc.tile_pool(name="casc", bufs=CASC_BUFS) as casc, \
             tc.tile_pool(name="wst", bufs=WST_BUFS) as wstp, \
             tc.tile_pool(name="ps", bufs=PT_BUFS + PC_BUFS,
                          space="PSUM") as ps:

            crA = sb.tile([P, ncols_a], BF16, tag="crA", name="crA")
            nc.sync.dma_start(crA[:], c_d[:, :ncols_a])
            crB = sb.tile([P, ncols_const - ncols_a], BF16, tag="crB",
                          name="crB")

            def cr(off, width=P):
                if off < ncols_a:
                    return crA[:, off:off + width]
                return crB[:, off - ncols_a:off - ncols_a + width]

            # PE warm-up: keep the tensor engine continuously busy from
            # ~0.3us so it reaches (and holds) full p-state before the
            # first real H-conv matmul. Reads a memset tile; outputs
            # rotate through the pt psum generations and are never read.
            if WARMUP_N:
                wt = sb.tile([P, P], BF16, tag="warm", name="warm")
                nc.vector.memset(wt[:], 0.0)
                for _ in range(WARMUP_N):
                    pw = ps.tile([P, 512] if not WIDE_PSUM else [P, W],
                                 F32, tag="pt", name="ptw", bufs=PT_BUFS)
                    nc.tensor.matmul(pw[:, :64], wt[:], wt[:, :64],
                                     start=True, stop=True)

            def wident(w):
                return cr(ident_offs[w])

            xq = []
            for g, n in enumerate(INPUT_GROUPS):
                t = sb.tile([P, n, W], BF16, tag=f"xq{g}", name=f"xq{g}")
                base = sum(INPUT_GROUPS[:g])
                nc.gpsimd.dma_start(
                    t[:],
                    bass.AP(x_d, base * P * W,
                            [[W, P], [P * W, n], [1, W]]))
                for k in range(n):
                    xq.append((t, k))
            # consts part B (levels 1-2 weights): HWDGE from SP, queued
            # behind the input loads so it rides the input-phase window
            nc.sync.dma_start(crB[:], c_d[:, ncols_a:])

            # c levels 0,1 in per-chunk tiles; level 2 staged in groups
            cbuf = [[sb.tile([P, W], BF16, tag=f"c{g}_{co}",
                             name=f"c{g}_{co}") for co in range(NCH)]
                    for g in range(LEVELS - 1)]
            c3t = {}

            def chunk_in(j, co, lo=0, hi=W):
                if j == 0:
                    t, k = xq[co]
                    return t[:, k, lo:hi]
                return cbuf[j - 1][co][:, lo:hi]

            def cur_ap(j, co, lo=0, hi=W):
                if j < LEVELS - 1:
                    return cbuf[j][co][:, lo:hi]
                base = 0
                for gi, sz in enumerate(C3_GROUP):
                    if co < base + sz:
                        break
                    base += sz
                if gi not in c3t:
                    c3t[gi] = wstp.tile([P, sz, W], BF16,
                                        tag="c3st", name="c3st",
                                        bufs=C3_BUFS)
                return c3t[gi][:, co - base, lo:hi]

            lags = (0, WAVE_LAG, 2 * WAVE_LAG) \
                if isinstance(WAVE_LAG, int) else \
                (0, WAVE_LAG[0], WAVE_LAG[1])
            order = []
            jorder = range(LEVELS - 1, -1, -1) if LEVEL_DESC \
                else range(LEVELS)
            for wave in range(NCH + lags[-1]):
                for j in jorder:
                    co = wave - lags[j]
                    if 0 <= co < NCH:
                        order.append((j, co))

            def gfind(parts, co):
                base = 0
                for gi, sz in enumerate(parts):
                    if co < base + sz:
                        return gi, co - base, sz, base
                    base += sz
                raise ValueError

            wsts = {}
            yxmap = {}

            def stageA(j, co):
                d = DILS[j]
                doffs, ooffs = cindex[j]
                on_pe = co in PE_ROUTE[j]

                yx = yxp.tile([P, WE], BF16, tag="yx", name="yx")
                yxmap[(j, co)] = yx
                mcopy = nc.scalar.copy if on_pe else nc.vector.tensor_copy
                if WIDE_PSUM:
                    pts = [ps.tile([P, W], F32, tag="pt", name="pt",
                                   bufs=PT_BUFS)]
                    dst = [(pts[0], 0), (pts[0], 512)]
                else:
                    pts = [ps.tile([P, 512], F32, tag="pt", name="pt",
                                   bufs=PT_BUFS) for _ in range(2)]
                    dst = [(pts[0], 0), (pts[1], 0)]
                for half in range(2):
                    lo, hi = half * 512, (half + 1) * 512
                    pt, po = dst[half]
                    mms = ([(doffs[co], None)] +
                           [(off, ci) for ci, off in ooffs[co]])
                    for i, (off, ci) in enumerate(mms):
                        nc.tensor.matmul(
                            pt[:, po:po + 512], cr(off),
                            chunk_in(j, co if ci is None else ci, lo, hi),
                            start=(i == 0),
                            stop=(i == len(mms) - 1))
                    nc.scalar.copy(
                        yx[:, MARG + lo:MARG + hi], pt[:, po:po + 512])
                    # reflected margin for this half, from yx (SBUF)
                    if half == 0:
                        mcopy(
                            bass.AP(yx.tensor, MARG - 2 * d,
                                    [[WE, P], [1, 2 * d]]),
                            bass.AP(yx.tensor, MARG + 2 * d,
                                    [[WE, P], [-1, 2 * d]]))
                    else:
                        mcopy(
                            bass.AP(yx.tensor, MARG + W,
                                    [[WE, P], [1, 2 * d]]),
                            bass.AP(yx.tensor, MARG + W - 2,
                                    [[WE, P], [-1, 2 * d]]))

            def stageB(j, co):
                d = DILS[j]
                on_pe = co in PE_ROUTE[j]
                yx = yxmap.pop((j, co))
                shifts = [(0, 6.0), (-d, 4.0), (d, 4.0),
                          (-2 * d, 1.0), (2 * d, 1.0)]

                # ---- W-conv ----
                if on_pe:
                    if WIDE_PSUM:
                        pcs = [ps.tile([P, W], F32, tag="pc", name="pc",
                                       bufs=PC_BUFS)]
                        cdst = [(pcs[0], 0), (pcs[0], 512)]
                    else:
                        pcs = [ps.tile([P, 512], F32, tag="pc",
                                       name="pc", bufs=PC_BUFS)
                               for _ in range(2)]
                        cdst = [(pcs[0], 0), (pcs[1], 0)]
                    for half in range(2):
                        lo, hi = half * 512, (half + 1) * 512
                        pc, po = cdst[half]
                        base = MARG + lo
                        for i, (off, wgt) in enumerate(shifts):
                            nc.tensor.matmul(
                                pc[:, po:po + 512],
                                wident(wgt),
                                bass.AP(yx.tensor, base + off,
                                        [[WE, P], [1, 512]]),
                                start=(i == 0),
                                stop=(i == len(shifts) - 1))
                        if not WIDE_PSUM:
                            nc.scalar.copy(cur_ap(j, co, lo, hi),
                                           pc[:, po:po + 512])
                    if WIDE_PSUM:
                        nc.scalar.copy(cur_ap(j, co), pcs[0][:])
                else:
                    def yxs(o, width):
                        return bass.AP(yx.tensor, MARG + o,
                                       [[WE, P], [1, width]])
                    t1 = casc.tile([P, WE], BF16, tag="t1", name="t1")
                    t2 = casc.tile([P, WE], BF16, tag="t2", name="t2")
                    fast = (j == 0 and co < FAST_CHUNKS)
                    if fast:
                        # split each stage L/R so the left half of c (and
                        # the sub/store after it) completes early
                        s1, s2, s3 = 512 + 3 * d, 512 + 2 * d, 512 + d
                        w1, w2, w3 = W + 3 * d, W + 2 * d, W + d
                        t3 = casc.tile([P, w3], BF16, tag="t3",
                                       name="t3")
                        # left half
                        nc.vector.tensor_add(
                            t1[:, :s1], yxs(-2 * d, s1), yxs(-d, s1))
                        nc.vector.tensor_add(
                            t2[:, :s2], t1[:, :s2], t1[:, d:d + s2])
                        nc.vector.tensor_add(
                            t3[:, :s3], t2[:, :s3], t2[:, d:d + s3])
                        nc.vector.tensor_add(
                            cur_ap(j, co, 0, 512),
                            t3[:, :512], t3[:, d:d + 512])
                        # right half
                        nc.vector.tensor_add(
                            t1[:, s1:w1], yxs(-2 * d + s1, w1 - s1),
                            yxs(-d + s1, w1 - s1))
                        nc.vector.tensor_add(
                            t2[:, s2:w2], t1[:, s2:w2],
                            t1[:, s2 + d:d + w2])
                        nc.vector.tensor_add(
                            t3[:, s3:w3], t2[:, s3:w3],
                            t2[:, s3 + d:d + w3])
                        nc.vector.tensor_add(
                            cur_ap(j, co, 512, W),
                            t3[:, 512:W], t3[:, 512 + d:W + d])
                    else:
                        w1 = W + 3 * d
                        e1 = nc.gpsimd if co in CASC1_POOL[j] \
                            else nc.vector
                        e1.tensor_add(
                            t1[:, :w1], yxs(-2 * d, w1), yxs(-d, w1))
                        w2 = W + 2 * d
                        nc.vector.tensor_add(
                            t2[:, :w2], t1[:, :w2], t1[:, d:d + w2])
                        w3 = W + d
                        nc.vector.tensor_add(
                            t1[:, :w3], t2[:, :w3], t2[:, d:d + w3])
                        nc.vector.tensor_add(
                            cur_ap(j, co), t1[:, :W], t1[:, d:d + W])

                # ---- fast path: half-split sub + per-half store ----
                # (the cascade above already produced cur_ap for non-split
                # chunks; fast chunks instead split the final sub+store in
                # halves so the left half ships ~1.5us earlier)
                if j == 0 and co < FAST_CHUNKS:
                    wf = wstp.tile([P, W], BF16, tag="wfast",
                                   name="wfast", bufs=2)
                    for half in range(2):
                        lo, hi = half * 512, (half + 1) * 512
                        nc.vector.tensor_sub(
                            wf[:, lo:hi], chunk_in(j, co, lo, hi),
                            cur_ap(j, co, lo, hi))
                        nc.gpsimd.dma_start(
                            bass.AP(out_d, j * H * W + co * P * W + lo,
                                    [[W, P], [1, 512]]),
                            wf[:, lo:hi])
                    return

                # ---- w_j = prev - cur (bf16) into staging ----
                fo = FAST_CHUNKS if j == 0 else 0
                hv, ci_, wg, wbase = gfind(W_GROUP[j], co - fo)
                wbase += fo
                if ci_ == 0:
                    wsts[(j, hv)] = wstp.tile([P, wg, W], BF16,
                                              tag="wst", name="wst")
                sube = nc.gpsimd if co in POOL_SUBS[j] else nc.vector
                sube.tensor_sub(
                    wsts[(j, hv)][:, ci_, :], chunk_in(j, co),
                    cur_ap(j, co))

                if ci_ == wg - 1:
                    nc.gpsimd.dma_start(
                        bass.AP(out_d, j * H * W + wbase * P * W,
                                [[W, P], [P * W, wg], [1, W]]),
                        wsts[(j, hv)][:])

                # ---- c3 group store (bf16 -> f32 cast DMA) ----
                if j == LEVELS - 1:
                    gv, cpos, csz, cbase = gfind(C3_GROUP, co)
                    if cpos == csz - 1:
                        nc.gpsimd.dma_start(
                            bass.AP(out_d, 3 * H * W + cbase * P * W,
                                    [[W, P], [P * W, csz], [1, W]]),
                            c3t[gv][:])

            if STAGE_LAG < 0:
                for j, co in order:
                    stageA(j, co)
                    stageB(j, co)
            else:
                # software-pipelined emission: stage B trails by STAGE_LAG
                # waves so every instruction is near-ready when its
                # (in-order) engine dispatches it
                waves = {}
                jorder2 = list(jorder)
                nwave = NCH + lags[-1]
                for wave in range(nwave):
                    lst = []
                    for j in jorder2:
                        co = wave - lags[j]
                        if 0 <= co < NCH:
                            lst.append((j, co))
                    waves[wave] = lst
                for wave in range(nwave + STAGE_LAG):
                    for j, co in waves.get(wave, ()):
                        stageA(j, co)
                    for j, co in waves.get(wave - STAGE_LAG, ()):
                        stageB(j, co)

    nc.compile()
    return nc, consts_np


_CACHE = {}


def _get_program():
    if "prog" not in _CACHE:
        _CACHE["prog"] = _build_program()
    return _CACHE["prog"]


def kernel(x, _trace=False, _trace_kwargs=None):
    """x: [8, 1024, 1024] float32 -> [8, 4, 1024, 1024] float32."""
    x = np.asarray(x)
    assert x.shape == (B, H, W) and x.dtype == np.float32
    nc, consts_np = _get_program()
    in_maps = [{"x": np.ascontiguousarray(x[b]), "consts": consts_np}
               for b in range(B)]
    kw = {}
    if _trace:
        kw = dict(trace=True, **(_trace_kwargs or {}))
    res = run_bass_kernel_spmd(nc, in_maps, core_ids=list(range(B)), **kw)
    out = np.stack([r["out"] for r in res.results], axis=0)
    if _trace:
        return out, res
    return out
